# revision 26
# baseline (speedup 1.0000x reference)
"""ALiBi transformer layer on 8 TRN2 NeuronCores.

Sharding: 2 batch groups x 4 cores. Core c (b=c//4, r=c%4) handles 4 heads
(Megatron column split) for batch b, then the 512-token slice r of out_proj
/ LN2 / FFN.

Key design points (v2):
  - All matmuls run in bf16 (same PE column rate as fp32r, but FWL weight
    loads, half the DMA, half the SBUF). PSUM accumulation stays fp32.
  - LN1 and LN2 are folded into the following matmuls: stats are computed
    with an all-ones [128,128] stationary (so mean/rstd land broadcast on
    all 128 partitions with no gpsimd broadcast), the -mu*colsum(W) rank-1
    term is added via a K=1 matmul row, and the *rstd scaling is applied to
    the matmul outputs.  Matmuls therefore never wait on a normalize pass.
  - Attention is software-pipelined per key-block: scores(kb+1) is issued
    before PV(kb), so the PE never idles while the scalar engine computes
    exp (keeps the HAM clock gate at 8/8).  Softmax uses no row-max: the
    alibi bias -slope*j keeps exp bounded, denominators come from an
    appended ones column in V.
  - Head outputs are redistributed with per-head-pair AllToAll (4x less
    wire than AllGather) in a layout where each received [128,512] block is
    exactly one out_proj contraction chunk - no indirect gather.
  - W1/W2 (bf16) are prefetched during attention / FFN1 respectively.
"""
import numpy as np
import ml_dtypes

import concourse.bass as bass
import concourse.tile as tile
import concourse.mybir as mybir
from concourse import bacc
from concourse.bass_utils import run_bass_kernel_spmd

B, S, D, H, DH, FF = 2, 2048, 1024, 16, 64, 4096
NCORES, GROUP = 8, 4
HPC = H // GROUP            # heads per core = 4
SL = S // GROUP             # token slice per core = 512
EPS = 1e-5
F32 = mybir.dt.float32
BF16 = mybir.dt.bfloat16
AF = mybir.ActivationFunctionType
ALU = mybir.AluOpType
ts = bass.ts


def build_nc(use_cqk: bool, use_cv: bool, use_c1: bool, use_b2: bool,
             dbg: bool = False):
    nc = bacc.Bacc("TRN2", target_bir_lowering=False, debug=False)

    srcTb = nc.dram_tensor("srcTb", (D, S), BF16, kind="ExternalInput")
    srcTs = nc.dram_tensor("srcTs", (D, SL), F32, kind="ExternalInput")
    wqkT = nc.dram_tensor("wqkT", (D, 2 * HPC * DH), BF16, kind="ExternalInput")
    wvT = nc.dram_tensor("wvT", (D, HPC * DH), BF16, kind="ExternalInput")
    woutT = nc.dram_tensor("woutT", (D, D), BF16, kind="ExternalInput")
    w1T = nc.dram_tensor("w1T", (D, FF), BF16, kind="ExternalInput")
    w2T = nc.dram_tensor("w2T", (FF, D), BF16, kind="ExternalInput")
    alibi = nc.dram_tensor("alibi", (128, HPC * 16), F32, kind="ExternalInput")
    tri = nc.dram_tensor("tri", (128, 128), BF16, kind="ExternalInput")
    csqk = nc.dram_tensor("csqk", (1, 2 * HPC * DH), BF16, kind="ExternalInput")
    csv = nc.dram_tensor("csv", (1, HPC * DH), F32, kind="ExternalInput")
    cs1 = nc.dram_tensor("cs1", (1, FF), BF16, kind="ExternalInput")
    cqk = nc.dram_tensor("cqk", (128, 4), F32, kind="ExternalInput")
    cv = nc.dram_tensor("cv", (1, HPC * DH), F32, kind="ExternalInput")
    c1 = nc.dram_tensor("c1", (128, 32), F32, kind="ExternalInput")
    b2c = nc.dram_tensor("b2c", (128, 8), F32, kind="ExternalInput")
    gidx = nc.dram_tensor("gidx", (128, 8), mybir.dt.int32, kind="ExternalInput")
    outT = nc.dram_tensor("outT", (D, SL), F32, kind="ExternalOutput")
    if dbg:
        d_qkT = nc.dram_tensor("d_qkT", (128, 4, S), BF16, kind="ExternalOutput")
        d_v = nc.dram_tensor("d_v", (128, 16, HPC, DH + 1), BF16,
                             kind="ExternalOutput")
        d_oh = nc.dram_tensor("d_oh", (DH + 1, HPC, S), BF16,
                              kind="ExternalOutput")
        d_ot = nc.dram_tensor("d_ot", (128, 8, SL), BF16, kind="ExternalOutput")
        d_src2 = nc.dram_tensor("d_src2", (128, 8, SL), F32,
                                kind="ExternalOutput")

    RG8 = [[0, 1, 2, 3, 4, 5, 6, 7]]

    with tile.TileContext(nc) as tc:
        # ---- persistent small constants ----
        consts = tc.alloc_tile_pool(name="consts", bufs=1)
        alibi_sb = consts.tile([128, HPC * 16], F32)
        nc.sync.dma_start(alibi_sb, alibi.ap())
        tri_sb = consts.tile([128, 128], BF16)
        nc.sync.dma_start(tri_sb, tri.ap())
        csqk_sb = consts.tile([1, 2 * HPC * DH], BF16)
        nc.sync.dma_start(csqk_sb, csqk.ap())
        cs1_sb = consts.tile([1, FF], BF16)
        nc.sync.dma_start(cs1_sb, cs1.ap())
        ones_sb = consts.tile([128, 128], BF16)
        nc.vector.memset(ones_sb, 1.0)
        eps_sb = consts.tile([1, 1], F32)
        nc.vector.memset(eps_sb, EPS)
        epsB_sb = consts.tile([128, 1], F32)
        nc.vector.memset(epsB_sb, EPS)
        csvB = consts.tile([128, HPC * DH], F32)
        csv_row = consts.tile([1, HPC * DH], F32)
        nc.sync.dma_start(csv_row, csv.ap())
        nc.gpsimd.partition_broadcast(csvB, csv_row)
        if use_cqk:
            cqk_sb = consts.tile([128, 4], F32)
            nc.sync.dma_start(cqk_sb, cqk.ap())
        if use_cv:
            cv_row = consts.tile([1, HPC * DH], F32)
            nc.sync.dma_start(cv_row, cv.ap())
            cvB = consts.tile([128, HPC * DH], F32)
            nc.gpsimd.partition_broadcast(cvB, cv_row)
        if use_c1:
            c1_sb = consts.tile([128, 32], F32)
            nc.sync.dma_start(c1_sb, c1.ap())
        if use_b2:
            b2_sb = consts.tile([128, 8], F32)
            nc.sync.dma_start(b2_sb, b2c.ap())
        gidx_sb = consts.tile([128, 8], mybir.dt.int32)
        nc.sync.dma_start(gidx_sb, gidx.ap())

        # qkT / v live from QKV until end of attention
        mids = tc.alloc_tile_pool(name="mids", bufs=1)
        qkT_sb = mids.tile([128, 4, S], BF16)        # [dh(2 heads), grp, t]
        v_sb = mids.tile([128, 16, HPC, DH + 1], BF16)  # [tok_p, tok_tile, h, dh|1]
        nc.gpsimd.memset(v_sb[:, :, :, DH:DH + 1], 1.0)

        # =============== P1: load + LN1 stats ===============
        p1big = tc.alloc_tile_pool(name="p1big", bufs=1)
        x_sb = p1big.tile([128, 8, S], BF16)
        srcT_v = srcTb.ap().rearrange("(g p) t -> p g t", p=128)
        for c in range(8):
            nc.sync.dma_start(x_sb[:, c, :], srcT_v[:, c, :])

        ln1 = tc.alloc_tile_pool(name="ln1", bufs=1)
        muB = ln1.tile([128, S], F32)       # mean, broadcast on all partitions
        rB = ln1.tile([128, S], F32)        # rstd, broadcast on all partitions
        nmu_bf = ln1.tile([1, S], BF16)     # -mu row (fold-matmul rhs)
        rc_cols = ln1.tile([128, 2, 16], F32)   # rstd / -mu as columns per tt
        rc_dram = tc.alloc_tile_pool(name="rcd", bufs=1, space="DRAM")
        rc_stage = rc_dram.tile([2, S], F32)
        # QKV weights: DMA queued right behind the src chunks
        wqk_sb = ln1.tile([128, 8, 2 * HPC * DH], BF16)
        nc.sync.dma_start(wqk_sb, wqkT.ap().rearrange("(g p) f -> p g f", p=128))
        wv_sb = ln1.tile([128, 8, HPC * DH], BF16)
        nc.sync.dma_start(wv_sb, wvT.ap().rearrange("(g p) f -> p g f", p=128))

        with (
            tc.tile_pool(name="p1sq", bufs=2) as p1sq,
            tc.tile_pool(name="p1ps", bufs=1, space="PSUM") as p1ps,
        ):
            st_x = p1ps.tile([128, 4, 512], F32)
            st_x2 = p1ps.tile([128, 4, 512], F32)
            sqs = [None] * 8
            # x2-stats lag one chunk so the PE never waits on a square op
            for c in range(9):
                if c < 8:
                    sq = p1sq.tile([128, S], BF16, tag="sq", name=f"sq{c}")
                    sqs[c] = sq
                    xs = x_sb[:, c, :]
                    if c % 2 == 0:
                        nc.vector.tensor_mul(sq, xs, xs)
                    else:
                        nc.scalar.activation(sq, xs, AF.Square)
                    for qg in range(4):
                        nc.tensor.matmul(st_x[:, qg, :], ones_sb,
                                         x_sb[:, c, ts(qg, 512)],
                                         start=(c == 0), stop=(c == 7))
                if c >= 1:
                    for qg in range(4):
                        nc.tensor.matmul(st_x2[:, qg, :], ones_sb,
                                         sqs[c - 1][:, ts(qg, 512)],
                                         start=(c == 1), stop=(c == 8))
            stx_flat = st_x.rearrange("p g t -> p (g t)")
            stx2_flat = st_x2.rearrange("p g t -> p (g t)")
            # muB released first so QKV matmuls can start sooner
            nc.vector.tensor_scalar_mul(muB, stx_flat, 1.0 / D)
            var = rB  # reuse storage
            nc.vector.scalar_tensor_tensor(var, muB, -1.0, muB,
                                           op0=ALU.mult, op1=ALU.mult)
            nc.vector.scalar_tensor_tensor(var, stx2_flat, 1.0 / D, var,
                                           op0=ALU.mult, op1=ALU.add)
        # rstd = exp(-0.5*ln(var+eps)); all on 128 partitions already
        nc.scalar.activation(rB, rB, AF.Ln, bias=epsB_sb, scale=1.0)
        nc.scalar.activation(rB, rB, AF.Exp, bias=0.0, scale=-0.5)
        nc.vector.tensor_scalar_mul(nmu_bf, muB[0:1, :], -1.0)
        # roundtrip rstd/-mu rows into token-on-partition columns (v post-op)
        nc.sync.dma_start(rc_stage[0:1, :], rB[0:1, :])
        nc.sync.dma_start(rc_stage[1:2, :], muB[0:1, :])
        nc.sync.dma_start(rc_cols,
                          rc_stage.rearrange("j (tt p) -> p j tt", p=128))

        # =============== P2: QKV (LN1 folded in) ===============
        with (
            tc.tile_pool(name="p2ps", bufs=3, space="PSUM") as p2ps,
            tc.tile_pool(name="p2psv", bufs=2, space="PSUM") as p2psv,
        ):
            for blk in [0, 2, 1, 3]:
                for qg in range(4):
                    qp = p2ps.tile([128, 512], F32, tag="qk")
                    for c in range(8):
                        nc.tensor.matmul(qp, wqk_sb[:, c, ts(blk, 128)],
                                         x_sb[:, c, ts(qg, 512)],
                                         start=(c == 0), stop=False)
                    # rank-1 fold: += (-mu) x colsum(W_blk)
                    nc.tensor.matmul(qp, csqk_sb[0:1, ts(blk, 128)],
                                     nmu_bf[0:1, ts(qg, 512)],
                                     start=False, stop=True)
                    dst = qkT_sb[:, blk, ts(qg, 512)]
                    nc.vector.tensor_mul(dst, qp, rB[:, ts(qg, 512)])
                    if use_cqk:
                        nc.gpsimd.tensor_scalar_add(dst, dst,
                                                    cqk_sb[:, blk:blk + 1])
            for tt in range(16):
                vp = p2psv.tile([128, HPC * DH], F32, tag="v")
                for c in range(8):
                    nc.tensor.matmul(vp, x_sb[:, c, ts(tt, 128)],
                                     wv_sb[:, c, :],
                                     start=(c == 0), stop=(c == 7))
                # v = (psum + (-mu_t)*colsum_v) * rstd_t (+ cv)
                vdst = v_sb[:, tt, :, 0:DH]
                csvB3 = csvB.rearrange("p (h d) -> p h d", h=HPC)
                vp3 = vp.rearrange("p (h d) -> p h d", h=HPC)
                nc.vector.scalar_tensor_tensor(
                    vdst, csvB3, rc_cols[:, 1, tt:tt + 1], vp3,
                    op0=ALU.mult, op1=ALU.add)
                if use_cv:
                    nc.gpsimd.scalar_tensor_tensor(
                        vdst, vdst, rc_cols[:, 0, tt:tt + 1],
                        cvB.rearrange("p (h d) -> p h d", h=HPC),
                        op0=ALU.mult, op1=ALU.add)
                else:
                    nc.gpsimd.tensor_scalar_mul(vdst, vdst,
                                                rc_cols[:, 0, tt:tt + 1])
        if dbg:
            nc.sync.dma_start(d_qkT.ap(), qkT_sb)
            nc.sync.dma_start(d_v.ap(), v_sb)
        rc_dram.release()
        ln1.release()
        p1big.release()

        # W1 prefetch (bf16, 8MB) - trickles in during attention
        pw1 = tc.alloc_tile_pool(name="pw1", bufs=1, side="right")
        w1_sb = pw1.tile([128, 8, FF], BF16)
        for c in range(8):
            nc.sync.dma_start(w1_sb[:, c, :],
                              w1T.ap().rearrange("(g p) f -> p g f", p=128)[:, c, :])
        # wout + residual-slice prefetch
        p4w = tc.alloc_tile_pool(name="p4w", bufs=1, side="right")
        wout_sb = p4w.tile([128, 8, D], BF16)
        nc.sync.dma_start(wout_sb, woutT.ap().rearrange("(g p) f -> p g f", p=128))
        srcTs_sb = p4w.tile([128, 8, SL], F32)
        nc.sync.dma_start(srcTs_sb, srcTs.ap().rearrange("(g p) t -> p g t", p=128))

        # =============== P3: attention ===============
        dram = tc.alloc_tile_pool(name="dram", bufs=1, space="DRAM")
        a2a_ins = [dram.tile([2, GROUP, 2, DH, SL], BF16, name=f"a2i{i}")
                   for i in range(2)]
        a2a_outs = [dram.tile([2 * GROUP, 128, SL], BF16, name=f"a2o{i}")
                    for i in range(2)]
        poh = tc.alloc_tile_pool(name="poh", bufs=1)
        # rows 0-63 oT, row 64 exp-sums (overwritten in place by 1/sum)
        oh_sb = poh.tile([DH + 1, HPC, S], BF16)

        with (
            tc.tile_pool(name="p3e", bufs=4) as p3e,
            tc.tile_pool(name="p3r", bufs=2) as p3r,
            tc.tile_pool(name="p3rb", bufs=2) as p3rb,
            tc.tile_pool(name="p3s", bufs=2, space="PSUM") as p3s,
            tc.tile_pool(name="p3pv", bufs=1, space="PSUM") as p3pv,
        ):
            for h in range(HPC):
                base = 64 * (h % 2)
                qgrp = h // 2
                kgrp = 2 + h // 2
                pvps = [p3pv.tile([DH + 1, 512], F32, tag=f"pv{i}",
                                  name=f"pv{h}_{i}")
                        for i in range(4)]
                ets = [None] * 16
                # software pipeline depth 2: scores/exp(kb), then PV(kb-2),
                # so the PE never waits on the scalar engine's exp
                for kb in range(18):
                    if kb < 16:
                        W = S - 128 * kb
                        et = p3e.tile([128, S], BF16, tag="et")
                        ets[kb] = et
                        kT = qkT_sb[base:base + 64, kgrp, ts(kb, 128)]
                        for sc in range((W + 1023) // 1024):
                            w = min(1024, W - 1024 * sc)
                            sp = p3s.tile([128, 1024], F32, tag="sc")
                            for half in range((w + 511) // 512):
                                ww = min(512, w - 512 * half)
                                off = 128 * kb + 1024 * sc + 512 * half
                                nc.tensor.matmul(
                                    sp[:, 512 * half:512 * half + ww],
                                    kT,
                                    qkT_sb[base:base + 64, qgrp, off:off + ww],
                                    start=True, stop=True)
                            nc.scalar.activation(
                                et[:, 1024 * sc:1024 * sc + w], sp[:, :w],
                                AF.Exp,
                                bias=alibi_sb[:, h * 16 + kb:h * 16 + kb + 1],
                                scale=0.125)
                        nc.vector.tensor_mul(et[:, 0:128], et[:, 0:128], tri_sb)
                    if kb >= 2:
                        pkb = kb - 2
                        pet = ets[pkb]
                        for qg in range(pkb // 4, 4):
                            ostart = max(0, 128 * pkb - 512 * qg)
                            estart = max(0, 512 * qg - 128 * pkb)
                            n = 512 - ostart
                            nc.tensor.matmul(
                                pvps[qg][:, ostart:512],
                                v_sb[:, pkb, h, :],
                                pet[:, estart:estart + n],
                                start=(pkb == 0), stop=(pkb == 4 * qg + 3))
                for qg in range(4):
                    nc.vector.tensor_copy(oh_sb[0:DH + 1, h, ts(qg, 512)],
                                          pvps[qg])

                if h % 2 == 1:
                    # normalize the pair and fire its AllToAll
                    pair = h // 2
                    # recip must land on a base-partition-0 tile:
                    # partition_broadcast replicates the tile's partition 0
                    rsum = p3r.tile([1, 2, S], BF16, tag="rs", name=f"rs{pair}")
                    nc.scalar.activation(
                        rsum, oh_sb[DH:DH + 1, 2 * pair:2 * pair + 2, :], AF.Ln)
                    nc.scalar.activation(rsum, rsum, AF.Exp, bias=0.0, scale=-1.0)
                    for hh in (0, 1):
                        rb = p3rb.tile([64, S], BF16, tag="rb",
                                       name=f"rb{pair}_{hh}")
                        nc.gpsimd.partition_broadcast(rb, rsum[:, hh, :],
                                                      channels=64)
                        eng = nc.vector if hh == 0 else nc.gpsimd
                        eng.tensor_mul(oh_sb[0:DH, 2 * pair + hh, :],
                                       oh_sb[0:DH, 2 * pair + hh, :], rb)
                    # write the same payload into both group-halves (SPMD:
                    # no core id available; peers outside the group discard)
                    for hh in (0, 1):
                        src_v = oh_sb[0:DH, 2 * pair + hh, :].rearrange(
                            "p (tb t) -> p tb t", tb=GROUP)
                        for gg in range(2):
                            nc.sync.dma_start(
                                a2a_ins[pair][gg, :, hh].rearrange(
                                    "tb p t -> p tb t"),
                                src_v)
                    nc.gpsimd.collective_compute(
                        "AllToAll", ALU.bypass,
                        replica_groups=RG8,
                        ins=[a2a_ins[pair].opt()],
                        outs=[a2a_outs[pair].opt()])

        if dbg:
            nc.sync.dma_start(d_oh.ap(), oh_sb)
        poh.release()
        mids.release()

        # =============== P4: out_proj + residual + LN2 stats ===============
        p46 = tc.alloc_tile_pool(name="p46", bufs=1)
        src2T_sb = p46.tile([128, 8, SL], F32)
        src2b_sb = p46.tile([128, 8, SL], BF16)
        ln2 = tc.alloc_tile_pool(name="ln2", bufs=1)
        r2B = ln2.tile([128, SL], F32)
        r2B_bf = ln2.tile([128, SL], BF16)
        nmu2_bf = ln2.tile([1, SL], BF16)
        with (
            tc.tile_pool(name="p4t", bufs=1) as p4t,
            tc.tile_pool(name="p4ps", bufs=2, space="PSUM") as p4ps,
            tc.tile_pool(name="p4ps2", bufs=1, space="PSUM") as p4ps2,
            tc.tile_pool(name="p4sq", bufs=2) as p4sq,
        ):
            ot_sb = p4t.tile([128, 8, SL], BF16)
            a2a_flats = [x.rearrange("s p t -> (s p) t") for x in a2a_outs]
            for c in [0, 2, 4, 6, 1, 3, 5, 7]:
                nc.gpsimd.indirect_dma_start(
                    out=ot_sb[:, c, :], out_offset=None, in_=a2a_flats[c % 2],
                    in_offset=bass.IndirectOffsetOnAxis(ap=gidx_sb[:, c:c + 1],
                                                        axis=0))
            st2x = p4ps2.tile([128, 512], F32)
            st2x2 = p4ps2.tile([128, 512], F32)
            # A2A0-dependent (even) chunks first per 4-block group: they run
            # while the pair-1 AllToAll is still in flight
            for grp in range(2):
                blks = range(4 * grp, 4 * grp + 4)
                ops = {blk: p4ps.tile([128, SL], F32, tag="op", bufs=4,
                                      name=f"op{blk}") for blk in blks}
                for blk in blks:
                    for c in [0, 2, 4, 6]:
                        nc.tensor.matmul(ops[blk], wout_sb[:, c, ts(blk, 128)],
                                         ot_sb[:, c, :],
                                         start=(c == 0), stop=False)
                for blk in blks:
                    for c in [1, 3, 5, 7]:
                        nc.tensor.matmul(ops[blk], wout_sb[:, c, ts(blk, 128)],
                                         ot_sb[:, c, :],
                                         start=False, stop=(c == 7))
                    nc.vector.tensor_add(src2T_sb[:, blk, :], ops[blk],
                                         srcTs_sb[:, blk, :])
                for blk in blks:
                    nc.gpsimd.tensor_copy(src2b_sb[:, blk, :],
                                          src2T_sb[:, blk, :])
                    sq2 = p4sq.tile([128, SL], BF16, tag="sq2")
                    nc.gpsimd.tensor_mul(sq2, src2b_sb[:, blk, :],
                                         src2b_sb[:, blk, :])
                    nc.tensor.matmul(st2x, ones_sb, src2b_sb[:, blk, :],
                                     start=(blk == 0), stop=(blk == 7))
                    nc.tensor.matmul(st2x2, ones_sb, sq2,
                                     start=(blk == 0), stop=(blk == 7))
            if dbg:
                nc.sync.dma_start(d_ot.ap(), ot_sb)
                nc.sync.dma_start(d_src2.ap(), src2T_sb)
            mu2 = r2B  # reuse
            nc.vector.tensor_scalar_mul(mu2, st2x, 1.0 / D)
            nc.vector.tensor_scalar_mul(nmu2_bf, mu2[0:1, :], -1.0)
            var2 = p4sq.tile([128, SL], F32, tag="v2", bufs=1)
            nc.vector.scalar_tensor_tensor(var2, mu2, -1.0, mu2,
                                           op0=ALU.mult, op1=ALU.mult)
            nc.vector.scalar_tensor_tensor(var2, st2x2, 1.0 / D, var2,
                                           op0=ALU.mult, op1=ALU.add)
            nc.scalar.activation(r2B, var2, AF.Ln, bias=epsB_sb, scale=1.0)
            nc.scalar.activation(r2B, r2B, AF.Exp, bias=0.0, scale=-0.5)
            nc.vector.tensor_copy(r2B_bf, r2B)
        p4w.release()

        # =============== P6: FFN (LN2 folded in) ===============
        with (
            tc.tile_pool(name="p6r", bufs=1) as p6r,
            tc.tile_pool(name="p6w", bufs=2) as p6w,
            tc.tile_pool(name="p6tmp", bufs=3) as p6tmp,
            tc.tile_pool(name="p6ps", bufs=3, space="PSUM") as p6ps,
            tc.tile_pool(name="p6ps2", bufs=2, space="PSUM") as p6ps2,
        ):
            relu_sb = p6r.tile([128, 32, SL], BF16)
            for fb in range(32):
                ps = p6ps.tile([128, SL], F32, tag="f1")
                for c in range(8):
                    nc.tensor.matmul(ps, w1_sb[:, c, ts(fb, 128)],
                                     src2b_sb[:, c, :],
                                     start=(c == 0), stop=False)
                nc.tensor.matmul(ps, cs1_sb[0:1, ts(fb, 128)], nmu2_bf,
                                 start=False, stop=True)
                tmp = p6tmp.tile([128, SL], BF16, tag="t1")
                if use_c1:
                    nc.scalar.activation(tmp, ps, AF.Relu,
                                         bias=c1_sb[:, fb:fb + 1])
                else:
                    nc.scalar.activation(tmp, ps, AF.Relu)
                nc.gpsimd.tensor_mul(relu_sb[:, fb, :], tmp, r2B_bf)
            outT_sb = p6r.tile([128, 8, SL], F32)
            for dblk in range(8):
                w2p = p6w.tile([128, 32, 128], BF16, tag="w2p")
                nc.sync.dma_start(
                    w2p,
                    w2T.ap()[:, ts(dblk, 128)].rearrange("(c p) d -> p c d", p=128))
                ps = p6ps2.tile([128, SL], F32, tag="f2")
                for c in range(32):
                    nc.tensor.matmul(ps, w2p[:, c, :],
                                     relu_sb[:, c, :],
                                     start=(c == 0), stop=(c == 31))
                if use_b2:
                    nc.vector.scalar_tensor_tensor(
                        outT_sb[:, dblk, :], ps, b2_sb[:, dblk:dblk + 1],
                        src2T_sb[:, dblk, :], op0=ALU.add, op1=ALU.add)
                else:
                    nc.vector.tensor_add(outT_sb[:, dblk, :], ps,
                                         src2T_sb[:, dblk, :])
                nc.sync.dma_start(
                    outT.ap().rearrange("(g p) t -> p g t", p=128)[:, dblk, :],
                    outT_sb[:, dblk, :])
        pw1.release()
        ln2.release()
        p46.release()
        dram.release()
        consts.release()

    nc.compile()
    return nc


_CACHE = {}


def _get_nc(flags):
    if flags not in _CACHE:
        _CACHE[flags] = build_nc(*flags)
    return _CACHE[flags]


def _bf16(a):
    return np.ascontiguousarray(a.astype(ml_dtypes.bfloat16))


def prep_in_maps(src, ln1_g, ln1_b, Wqkv, bqkv, Wout, bout, ln2_g, ln2_b,
                 W1, b1, W2, b2):
    src = np.asarray(src, np.float32)
    ln1_g = np.asarray(ln1_g, np.float32); ln1_b = np.asarray(ln1_b, np.float32)
    Wqkv = np.asarray(Wqkv, np.float32); bqkv = np.asarray(bqkv, np.float32)
    Wout = np.asarray(Wout, np.float32); bout = np.asarray(bout, np.float32)
    ln2_g = np.asarray(ln2_g, np.float32); ln2_b = np.asarray(ln2_b, np.float32)
    W1 = np.asarray(W1, np.float32); b1 = np.asarray(b1, np.float32)
    W2 = np.asarray(W2, np.float32); b2 = np.asarray(b2, np.float32)

    WqkvT_g = ln1_g[:, None] * Wqkv.T          # [D, 3D]
    const_qkv = ln1_b @ Wqkv.T + bqkv          # [3D]
    w1T_g = ln2_g[:, None] * W1.T              # [D, FF]
    c1_full = ln2_b @ W1.T + b1                # [FF]
    w2T = _bf16(W2.T)                          # [FF, D]
    woutT = _bf16(Wout.T)                      # [D, D]
    cs1_row = _bf16(w1T_g.sum(axis=0).reshape(1, FF))
    b2col = np.ascontiguousarray(b2.reshape(8, 128).T)        # [128, 8]
    c1col = np.ascontiguousarray(c1_full.reshape(32, 128).T)  # [128, 32]

    tri_np = (np.arange(128)[:, None] <= np.arange(128)[None, :])
    tri_np = _bf16(tri_np.astype(np.float32))

    use_c1 = bool(np.any(c1_full))
    use_b2 = bool(np.any(b2))

    in_maps = []
    use_cqk = use_cv = False
    for c in range(NCORES):
        b, r = c // GROUP, c % GROUP
        heads = list(range(HPC * r, HPC * r + HPC))
        qcols = np.concatenate([np.arange(DH * h, DH * h + DH) for h in heads])
        kcols = qcols + D
        vcols = qcols + 2 * D
        qkcols = np.concatenate([qcols, kcols])
        srcTb_np = _bf16(src[b].T)
        srcTs_np = np.ascontiguousarray(src[b].T[:, SL * r:SL * r + SL]
                                        + bout[:, None])
        wqkT_np = _bf16(WqkvT_g[:, qkcols])
        wvT_np = _bf16(WqkvT_g[:, vcols])
        csqk_np = _bf16(WqkvT_g[:, qkcols].sum(axis=0).reshape(1, -1))
        # negated: the v post-op computes (csv * mu) + psum with mu (not -mu)
        csv_np = np.ascontiguousarray(
            -WqkvT_g[:, vcols].sum(axis=0).reshape(1, -1))
        cqk_np = np.ascontiguousarray(const_qkv[qkcols].reshape(4, 128).T)
        cv_np = np.ascontiguousarray(const_qkv[vcols].reshape(1, HPC * DH))
        if np.any(cqk_np):
            use_cqk = True
        if np.any(cv_np):
            use_cv = True
        ali = np.zeros((128, HPC * 16), np.float32)
        j = np.arange(128)
        for hl, hh in enumerate(heads):
            slope = 2.0 ** (-float(hh))
            for kb in range(16):
                ali[:, hl * 16 + kb] = -slope * (kb * 128 + j)
        # receive-side gather: chunk c' <- a2a block s = 4*g + c'//2
        g = c // GROUP
        gidx_np = np.empty((128, 8), np.int32)
        for cp in range(8):
            gidx_np[:, cp] = 128 * (4 * g + cp // 2) + np.arange(128)
        in_maps.append(dict(
            srcTb=srcTb_np, srcTs=srcTs_np, wqkT=wqkT_np, wvT=wvT_np,
            woutT=woutT, w1T=_bf16(w1T_g), w2T=w2T,
            alibi=np.ascontiguousarray(ali), tri=tri_np,
            csqk=csqk_np, csv=csv_np, cs1=cs1_row,
            cqk=cqk_np, cv=cv_np, c1=c1col, b2c=b2col,
            gidx=np.ascontiguousarray(gidx_np)))

    return in_maps, (use_cqk, use_cv, use_c1, use_b2)


def kernel(**inputs):
    _want_trace = inputs.pop("_want_trace", False)
    in_maps, flags = prep_in_maps(**inputs)
    nc = _get_nc(flags)
    # filter to the inputs the compiled program actually kept
    expected = set()
    for alloc in nc.m.functions[0].allocations:
        if isinstance(alloc, mybir.MemoryLocationSet) and \
           alloc.kind == "ExternalInput":
            expected.add(alloc.memorylocations[0].name)
    in_maps = [{k: v for k, v in m.items() if k in expected} for m in in_maps]

    res = run_bass_kernel_spmd(nc, in_maps, core_ids=list(range(NCORES)),
                               trace=_want_trace)
    out = np.empty((B, S, D), np.float32)
    for c in range(NCORES):
        b, r = c // GROUP, c % GROUP
        out[b, SL * r:SL * r + SL, :] = res.results[c]["outT"].T
    if _want_trace:
        return out, res
    return out
